# revision 80
# baseline (speedup 1.0000x reference)
"""Trainium2 Bass kernel for GQA attention forward (B=2, S=2048, D=2048,
16 q-heads / 4 kv-heads, head_dim=128, RoPE, causal).

Sharding: 8 cores = 2 (batch) x 4 (kv-head groups).  Each core computes its
batch's attention for one kv-head group (4 q-heads + 1 kv head) and a
row-parallel partial of the output projection; the host sums the 4 bf16
partials per batch.

Precision: the q/k/v and output projections run as fp8e4 DoubleRow matmuls
(2 contraction k-tiles per instruction at 0.5 cyc/row = 4x bf16 MACs).
Each operand is split hi/lo: hi = fp8(s*a), lo = fp8(s*a - hi), with the
scale s chosen so residuals stay above e4m3's subnormal floor.  A 3-term
product (hh + hl + lh) then costs 0.75x bf16 with ~0.1% error (the dropped
lo*lo term is O(eps^2)).  x and the weights split host-side for free; the
attention-output slabs split on-device (1 ACT copy + 1 DVE sub per slab).
QK and PV stay bf16 (splitting q/k/p on device would cost more elementwise
work than the PE saving).

Scale bookkeeping: x*16, w*512 -> proj psum = 8192*val.  The RoPE tables
are pre-divided by 8192 so RoPE reads the psum directly; the v copy folds
*32/8192 so v_sb = 32*v.  ps_o = 32*o, so the softmax-normalized oT32 =
32*o fits fp8 comfortably; wo*512 -> out psum = 16384*out, unscaled in the
PSUM->SBUF copies.

Design (all phases software-pipelined over four 512-row s-chunks):
  * q/k projections are emitted directly in [e, s] (transposed) form
    (lhsT = weight tile, rhs = xT tile) -- no PE transposes anywhere.
  * RoPE runs on DVE straight from the projection psum (PSUM operands are
    exempt from walrus' same-start-partition rule, so the half-swap rides
    the psum read), with the unscale folded into the tables.
  * Scores keep keys in partitions / queries free, so exp output feeds the
    PV matmul directly.  Only the 128-wide sub-diagonal block needs
    masking: a 0/1 lower-triangle multiply on the idle Pool engine after
    exp (columns are realigned so it is always the first written block).
  * The softmax denominator is a bf16 pair tree on DVE over the exp tiles,
    then a 128-partition sum + reciprocal broadcast on the idle Pool
    engine (partition_all_reduce) -- the tensor engine never touches it.
  * Emission order: B(c)'s head windows are exp-throughput-bound, so each
    interleaves BOTH the previous chunk's output-projection slab C(c-1)
    AND the NEXT chunk's k/q projection chains A(c+1) after each head --
    the PE never idles on the ACT exp chain in steady state.  Only the
    v-chains remain in the standalone A slot between chunks.
  * The final chunk's heads 2-3 skip the fp8 split and their out-proj
    terms run as bf16 matmuls (wob), so the drain never waits on the
    last norm's fp8 split chain.
  * x streams per-chunk on the sync queue in a few coarse pieces (each
    dma_start costs ~565ns dispatch + 900ns completion-semaphore, so
    fewer/bigger transfers win); weights (head-major for wq, k-columns
    before v-columns) and RoPE tables ride the scalar queue.
PSUM budget is exactly 8 banks: A-chains/diag-scores/out-proj 3,
scores 3, v/pv 2.
"""

import sys

if "/opt/trn_rl_repo" not in sys.path:
    sys.path.insert(0, "/opt/trn_rl_repo")

import numpy as np
import ml_dtypes

import concourse.bass as bass
import concourse.bass_isa as bass_isa
import concourse.tile as tile
from concourse import mybir

F32 = mybir.dt.float32
F32R = mybir.dt.float32r
BF16 = mybir.dt.bfloat16
FP8 = mybir.dt.float8e4
DR = mybir.MatmulPerfMode.DoubleRow

# Full-problem constants (per reference).
B, S, DIM = 2, 2048, 2048
N_HEADS, N_KV_HEADS, HEAD_DIM = 16, 4, 128
N_GROUPS = N_KV_HEADS          # tensor-parallel groups
HQ = N_HEADS // N_KV_HEADS     # q heads per group

# hi/lo fp8 scales (powers of two; residuals stay in e4m3 range)
SX = 16.0       # x
SW = 512.0      # wq/wk/wv/wo
SV = 32.0       # v_sb = 32*v  (also oT32 = 32*o)
INV_PROJ = 1.0 / (SX * SW)          # q/k psum unscale (folded into tables)
INV_V = SV / (SX * SW)              # v psum -> 32*v
INV_OUT = 1.0 / (SV * SW)           # out-proj psum unscale


def build_attention_core(nc, S=S, D=DIM, HQ=HQ, HD=HEAD_DIM, CHUNK=512):
    n_st = S // 128        # s tiles
    n_dt = D // 128        # d tiles
    n_ch = S // CHUNK      # s chunks
    kpc = CHUNK // 128     # k-tiles per chunk
    n_dc = D // CHUNK      # d chunks (phase C)
    spc = CHUNK // 128     # s-tiles per chunk
    n_pr = n_dt // 2       # contraction k-tile pairs for projections
    IQ = HQ * HD

    xh_d = nc.dram_tensor("xTh", [128, n_dt, S], FP8, kind="ExternalInput")
    xl_d = nc.dram_tensor("xTl", [128, n_dt, S], FP8, kind="ExternalInput")
    wqh_d = nc.dram_tensor("wqTh", [128, HQ, n_dt, HD], FP8,
                           kind="ExternalInput")
    wql_d = nc.dram_tensor("wqTl", [128, HQ, n_dt, HD], FP8,
                           kind="ExternalInput")
    wkh_d = nc.dram_tensor("wkTh", [128, n_dt, HD], FP8,
                           kind="ExternalInput")
    wkl_d = nc.dram_tensor("wkTl", [128, n_dt, HD], FP8,
                           kind="ExternalInput")
    wvh_d = nc.dram_tensor("wvTh", [128, n_dt, HD], FP8,
                           kind="ExternalInput")
    wvl_d = nc.dram_tensor("wvTl", [128, n_dt, HD], FP8,
                           kind="ExternalInput")
    woh_d = nc.dram_tensor("woTh", [128, IQ // 128, D], FP8,
                           kind="ExternalInput")
    wol_d = nc.dram_tensor("woTl", [128, IQ // 128, D], FP8,
                           kind="ExternalInput")
    wob_d = nc.dram_tensor("woTb", [128, 2, D], BF16, kind="ExternalInput")
    t1_d = nc.dram_tensor("t1", [128, S], BF16, kind="ExternalInput")
    t2_d = nc.dram_tensor("t2", [128, S], BF16, kind="ExternalInput")
    tri_d = nc.dram_tensor("tri", [128, 128], BF16, kind="ExternalInput")
    out_d = nc.dram_tensor("out_partial", [S, D], BF16, kind="ExternalOutput")

    scale = float(HD) ** -0.5

    def mm3(ps, lh, ll, rh, rl, first, last):
        """3-term hi/lo fp8 DoubleRow product into psum ps.

        lh/ll, rh/rl: [128, 2, M] / [128, 2, N] k-tile-pair slices.
        Terms: hh + hl + lh (lo*lo dropped, O(eps^2))."""
        nc.tensor.matmul(ps, lh, rh, start=first, stop=False, perf_mode=DR)
        nc.tensor.matmul(ps, lh, rl, start=False, stop=False, perf_mode=DR)
        nc.tensor.matmul(ps, ll, rh, start=False, stop=last, perf_mode=DR)

    with tile.TileContext(nc) as tc:
        with (
            tc.tile_pool(name="persist", bufs=1) as persist,
            tc.tile_pool(name="xin", bufs=1) as xin_pool,
            tc.tile_pool(name="rope", bufs=4) as rope_pool,
            tc.tile_pool(name="expt", bufs=10) as expt_pool,
            tc.tile_pool(name="acc", bufs=3) as acc_pool,
            tc.tile_pool(name="pairs", bufs=4) as pair_pool,
            tc.tile_pool(name="recip", bufs=3) as rec_pool,
            tc.tile_pool(name="norm", bufs=3) as norm_pool,
            tc.tile_pool(name="outsb", bufs=4) as outsb_pool,
            # PSUM: 8 banks total
            tc.tile_pool(name="ps_a", bufs=3, space="PSUM") as psa_pool,   # 3
            tc.tile_pool(name="ps_s", bufs=3, space="PSUM") as pss_pool,   # 3
            tc.tile_pool(name="ps_o", bufs=2, space="PSUM") as pso_pool,   # 2
        ):
            # ---------------- weights + constants ---------------------------
            wqh_sb = persist.tile([128, HQ, n_dt, HD], FP8)
            wql_sb = persist.tile([128, HQ, n_dt, HD], FP8)
            wkh_sb = persist.tile([128, n_dt, HD], FP8)
            wkl_sb = persist.tile([128, n_dt, HD], FP8)
            wvh_sb = persist.tile([128, n_dt, HD], FP8)
            wvl_sb = persist.tile([128, n_dt, HD], FP8)
            t1_sb = persist.tile([128, S], BF16)
            t2_sb = persist.tile([128, S], BF16)
            # coarse weight pieces (each dispatch ~650ns of queue time):
            # small wk lead so the k-chain starts early, then the rest;
            # the v-columns load separately after the q weights
            for gs in (slice(0, 2), slice(2, n_dt)):
                nc.scalar.dma_start(out=wkh_sb[:, gs, :], in_=wkh_d[:, gs, :])
                nc.scalar.dma_start(out=wkl_sb[:, gs, :], in_=wkl_d[:, gs, :])
                if gs.start == 0:
                    # chunk-0 RoPE tables early: k-rope needs them ~6us in
                    c0s = slice(0, CHUNK)
                    nc.scalar.dma_start(out=t1_sb[:, c0s], in_=t1_d[:, c0s])
                    nc.scalar.dma_start(out=t2_sb[:, c0s], in_=t2_d[:, c0s])
            for h in range(HQ):
                # head-major: q-chain h can start as soon as its slab lands
                # (head 0's lo slab rides the sync queue right after x0's
                # lead so the first q-chain is not gated on this queue)
                nc.scalar.dma_start(out=wqh_sb[:, h, :, :], in_=wqh_d[:, h, :, :])
                if h > 1:
                    nc.scalar.dma_start(out=wql_sb[:, h, :, :],
                                        in_=wql_d[:, h, :, :])
            rst = slice(CHUNK, S)
            nc.scalar.dma_start(out=t1_sb[:, rst], in_=t1_d[:, rst])
            nc.scalar.dma_start(out=t2_sb[:, rst], in_=t2_d[:, rst])
            nc.scalar.dma_start(out=wvh_sb, in_=wvh_d[:])
            nc.scalar.dma_start(out=wvl_sb, in_=wvl_d[:])
            tri_sb = persist.tile([128, 128], BF16)
            nc.scalar.dma_start(out=tri_sb, in_=tri_d[:])
            woh_sb = persist.tile([128, IQ // 128, D], FP8)
            wol_sb = persist.tile([128, IQ // 128, D], FP8)
            nc.scalar.dma_start(out=woh_sb, in_=woh_d[:])
            nc.scalar.dma_start(out=wol_sb, in_=wol_d[:])
            # bf16 wo rows for the last chunk's heads 2-3: their out-proj
            # terms run in bf16 so the tail never waits on an fp8 split
            wob_sb = persist.tile([128, 2, D], BF16)
            nc.scalar.dma_start(out=wob_sb, in_=wob_d[:])

            # x streamed per s-chunk on the sync queue (hi+lo interleaved)
            x_ch = []
            for c in range(n_ch):
                xh = xin_pool.tile([128, n_dt, CHUNK], FP8, tag=f"xh{c % 2}",
                                   name=f"xh{c}")
                xl = xin_pool.tile([128, n_dt, CHUNK], FP8, tag=f"xl{c % 2}",
                                   name=f"xl{c}")
                c_sl = slice(c * CHUNK, (c + 1) * CHUNK)
                # coarse pieces: each dma_start costs ~650ns of sync-queue
                # dispatch, so fewer/bigger transfers win; chunk 0 leads
                # with a small piece so the first k-chain pairs start early
                pieces = [(0, 2), (2, 8), (8, 16)] if c == 0 else [(0, 8), (8, 16)]
                for lo_, hi_ in pieces:
                    gs = slice(lo_, hi_)
                    nc.sync.dma_start(out=xh[:, gs, :], in_=xh_d[:, gs, c_sl])
                    nc.sync.dma_start(out=xl[:, gs, :], in_=xl_d[:, gs, c_sl])
                if c == 0:
                    for hq_ in range(2):
                        nc.sync.dma_start(out=wql_sb[:, hq_, :, :],
                                          in_=wql_d[:, hq_, :, :])
                x_ch.append((xh, xl))

            # persistent activations
            qT_sb = persist.tile([128, HQ, S], BF16)    # [e, h, s]
            kT_sb = persist.tile([128, S], BF16)        # [e, s]
            v_sb = persist.tile([128, n_st, HD], BF16)  # [s_in_tile, s_tile, e] (32*v)
            oTh_sb = persist.tile([128, HQ, S], FP8)    # 32*o hi
            oTl_sb = persist.tile([128, HQ, S], FP8)    # 32*o lo
            oT32_sb = persist.tile([128, 2, CHUNK], BF16)  # last-chunk h2/h3

            # deferred per-(h,c) normalization tail (keeps the Pool/DVE
            # queues clear of slow norm ops until the next head's first
            # mask/pair ops have been emitted)
            norm_pending = [None]

            def emit_norm():
                acc_, ps_o_, h_, c_ = norm_pending[0]
                norm_pending[0] = None
                # softmax denominator: 128-partition sum of the bf16 pair
                # tree, broadcast to all partitions, on the idle Pool engine
                sum_sb = rec_pool.tile([128, CHUNK], F32, tag="sum_sb")
                nc.gpsimd.partition_all_reduce(
                    sum_sb, acc_, channels=128, reduce_op=bass_isa.ReduceOp.add
                )
                rec_sb = rec_pool.tile([128, CHUNK], F32, tag="rec_sb")
                nc.vector.reciprocal_approx_fast(rec_sb, sum_sb)
                if c_ == n_ch - 1 and h_ >= 2:
                    # tail heads stay bf16 (consumed by bf16 matmuls in the
                    # final out-proj) -- no fp8 split on the critical tail
                    nc.vector.tensor_mul(oT32_sb[:, h_ - 2, :], ps_o_, rec_sb)
                    return
                o32 = norm_pool.tile([128, CHUNK], BF16, tag="o32")
                nc.vector.tensor_mul(o32, ps_o_, rec_sb)
                sl = slice(c_ * CHUNK, (c_ + 1) * CHUNK)
                # hi/lo fp8 split of the 32*o slab for the DoubleRow out-proj
                nc.scalar.copy(oTh_sb[:, h_, sl], o32)
                nc.vector.tensor_sub(oTl_sb[:, h_, sl], o32, oTh_sb[:, h_, sl])

            def emit_out_tile(c, sj, last=False):
                """Phase C for s-tile sj of chunk c: one 128-row output slab.
                Interleaved into B(c+1)'s head loop: its matmuls fill the PE
                gaps where B is exp-throughput-bound, and its PSUM->SBUF
                copies ride whichever of ACT/DVE has slack in that window."""
                st = c * spc + sj
                s128 = slice(st * 128, (st + 1) * 128)
                row_sb = outsb_pool.tile([128, D], BF16, tag="out_sb")
                for dc in range(n_dc):
                    if last and dc % 2 == 1:
                        ps_d = pss_pool.tile([128, CHUNK], F32, tag="ps_s")
                    else:
                        ps_d = psa_pool.tile([128, CHUNK], F32, tag="ps_a")
                    d_sl = slice(dc * CHUNK, (dc + 1) * CHUNK)
                    if last:
                        sjc = slice(sj * 128, (sj + 1) * 128)
                        it = slice(0, 2)
                        mm3(ps_d,
                            oTh_sb[:, it, s128], oTl_sb[:, it, s128],
                            woh_sb[:, it, d_sl], wol_sb[:, it, d_sl],
                            first=True, last=False)
                        for hh in range(2):
                            nc.tensor.matmul(
                                ps_d, oT32_sb[:, hh, sjc],
                                wob_sb[:, hh, d_sl],
                                start=False, stop=(hh == 1))
                    else:
                        for t in range(IQ // 256):   # it-pairs
                            it = slice(2 * t, 2 * t + 2)
                            mm3(ps_d,
                                oTh_sb[:, it, s128], oTl_sb[:, it, s128],
                                woh_sb[:, it, d_sl], wol_sb[:, it, d_sl],
                                first=(t == 0), last=(t == IQ // 256 - 1))
                    dst = row_sb[:, d_sl]
                    # engine choice tracks which engine has slack in the
                    # B window this chunk interleaves with (ACT saturates
                    # as c grows; DVE is flatter)
                    if c == 0:
                        use_act = True
                    elif c == 1:
                        use_act = dc % 2 == 0
                    elif c == 2:
                        use_act = False
                    else:
                        use_act = dc % 2 == 0
                    if use_act:
                        nc.scalar.mul(dst, ps_d, INV_OUT)
                    else:
                        nc.vector.tensor_scalar_mul(dst, ps_d, INV_OUT)
                if last and sj == spc - 1:
                    # final tile: split the store so the tail drains as the
                    # copies complete instead of after the whole row
                    for dc in range(n_dc):
                        nc.sync.dma_start(
                            out=out_d[st * 128:(st + 1) * 128,
                                      dc * CHUNK:(dc + 1) * CHUNK],
                            in_=row_sb[:, dc * CHUNK:(dc + 1) * CHUNK],
                        )
                else:
                    nc.sync.dma_start(
                        out=out_d[st * 128:(st + 1) * 128, :], in_=row_sb
                    )

            def rope(dst, src, c):
                """dst[e, s-chunk] = src*t1 + swap_half(src)*t2, read straight
                from the projection psum (tables carry the 1/8192 unscale;
                psum operands may start at partition 64 -- the SBUF
                same-start-partition rule does not apply)."""
                c_sl = slice(c * CHUNK, (c + 1) * CHUNK)
                t1c = t1_sb[:, c_sl]
                t2c = t2_sb[:, c_sl]
                m1 = rope_pool.tile([128, CHUNK], BF16, tag="m1")
                nc.vector.tensor_mul(m1, src, t1c)
                m2 = rope_pool.tile([128, CHUNK], BF16, tag="m2")
                nc.vector.tensor_mul(m2[0:64, :], src[64:128, :], t2c[0:64, :])
                nc.vector.tensor_mul(m2[64:128, :], src[0:64, :], t2c[64:128, :])
                nc.vector.tensor_add(dst, m1, m2)

            def emit_k_chain(c):
                c_sl = slice(c * CHUNK, (c + 1) * CHUNK)
                xh, xl = x_ch[c]
                ps_k = psa_pool.tile([128, CHUNK], F32, tag="ps_a")
                for t in range(n_pr):
                    dt2 = slice(2 * t, 2 * t + 2)
                    mm3(ps_k,
                        wkh_sb[:, dt2, :], wkl_sb[:, dt2, :],
                        xh[:, dt2, :], xl[:, dt2, :],
                        first=(t == 0), last=(t == n_pr - 1))
                if norm_pending[0] is not None:
                    emit_norm()
                rope(kT_sb[:, c_sl], ps_k, c)

            def emit_q_chain(c, h):
                c_sl = slice(c * CHUNK, (c + 1) * CHUNK)
                xh, xl = x_ch[c]
                ps_qh = psa_pool.tile([128, CHUNK], F32, tag="ps_a")
                for t in range(n_pr):
                    dt2 = slice(2 * t, 2 * t + 2)
                    mm3(ps_qh,
                        wqh_sb[:, h, dt2, :], wql_sb[:, h, dt2, :],
                        xh[:, dt2, :], xl[:, dt2, :],
                        first=(t == 0), last=(t == n_pr - 1))
                rope(qT_sb[:, h, c_sl], ps_qh, c)

            def emit_v_chains(c):
                # v: natural [s, e] layout, one chain per s-tile; the
                # four chains share one bank from the ps_o rotation
                xh, xl = x_ch[c]
                ps_vt = pso_pool.tile([128, CHUNK], F32, tag="o")
                for sj in range(spc):
                    st = c * spc + sj
                    sj_sl = slice(sj * 128, (sj + 1) * 128)
                    for t in range(n_pr):
                        dt2 = slice(2 * t, 2 * t + 2)
                        mm3(ps_vt[:, sj_sl],
                            xh[:, dt2, sj_sl], xl[:, dt2, sj_sl],
                            wvh_sb[:, dt2, :], wvl_sb[:, dt2, :],
                            first=(t == 0), last=(t == n_pr - 1))
                    nc.scalar.mul(v_sb[:, st, :], ps_vt[:, sj_sl], INV_V)

            for c in range(n_ch):
                c_sl = slice(c * CHUNK, (c + 1) * CHUNK)

                # ======== A(c): projections + RoPE ========
                # each chunk's k/q chains (except chunk 0's) are interleaved
                # into the previous chunk's B head windows, which are
                # exp-paced with idle PE; only the v-chains remain here
                if c == 0:
                    emit_k_chain(c)
                    for h in range(HQ):
                        emit_q_chain(c, h)
                emit_v_chains(c)

                # ======== B(*, c): attention for q-chunk c ========
                for h in range(HQ):
                    ps_o = pso_pool.tile([128, CHUNK], F32, tag="o")
                    n_kj = (c + 1) * kpc
                    acc = acc_pool.tile([128, CHUNK], BF16, tag="acc")
                    pend_pv = []
                    stash_exp = [None]
                    stash_pair = [None]
                    n_acc = [0]

                    def flush_pv():
                        pe_, pj, poff = pend_pv.pop(0)
                        nc.tensor.matmul(
                            ps_o[:, poff:], v_sb[:, pj, :], pe_,
                            start=(pj == 0), stop=(pj == n_kj - 1),
                        )

                    for kj in range(n_kj):
                        off = max(0, (kj - c * kpc)) * 128
                        w = CHUNK - off
                        ps_s = pss_pool.tile([128, CHUNK], F32, tag="ps_s")
                        nc.tensor.matmul(
                            ps_s[:, 0:w],
                            kT_sb[:, kj * 128:(kj + 1) * 128],
                            qT_sb[:, h, c * CHUNK + off:(c + 1) * CHUNK],
                            start=True, stop=True,
                        )
                        if kj == 3 and norm_pending[0] is not None:
                            emit_norm()
                        expT = expt_pool.tile([128, CHUNK], BF16, tag="expT")
                        if off > 0:
                            # exp output is realigned to q-in-chunk columns;
                            # zero the fully-masked leading columns so the
                            # denominator tree can run full-width
                            nc.gpsimd.memset(expT[:, 0:off], 0.0)
                        nc.scalar.activation(
                            expT[:, off:], ps_s[:, 0:w],
                            mybir.ActivationFunctionType.Exp,
                            scale=scale,
                        )
                        if kj >= c * kpc:
                            # causal mask: exp column off+i holds q-position
                            # off+i and partition p holds k-position off+p,
                            # so the 0/1 lower triangle masks the first
                            # written 128 columns (on Pool; exp overflow is
                            # not a risk at these score magnitudes)
                            nc.gpsimd.tensor_mul(
                                expT[:, off:off + 128],
                                expT[:, off:off + 128], tri_sb,
                            )
                        pend_pv.append((expT[:, off:], kj, off))
                        if len(pend_pv) > 2:
                            flush_pv()
                        # denominator: bf16 pair tree on DVE (full width --
                        # masked regions of expT are zeroed above)
                        if kj % 2 == 0:
                            stash_exp[0] = expT
                        else:
                            pr = pair_pool.tile([128, CHUNK], BF16, tag="pair")
                            nc.vector.tensor_add(pr, stash_exp[0], expT)
                            stash_exp[0] = None
                            if n_acc[0] == 0 and stash_pair[0] is None:
                                stash_pair[0] = pr
                            elif n_acc[0] == 0:
                                nc.vector.tensor_add(acc, stash_pair[0], pr)
                                stash_pair[0] = None
                                n_acc[0] = 1
                            else:
                                nc.vector.tensor_add(acc, acc, pr)
                                n_acc[0] += 1
                    while pend_pv:
                        flush_pv()
                    # n_kj is always >= 4 so at least two pairs were formed
                    # and acc is initialized by the second pair.
                    assert n_acc[0] >= 1
                    norm_pending[0] = (acc, ps_o, h, c)
                    if c + 1 < n_ch:
                        # fill B(c)'s exp-paced windows with A(c+1)'s k/q
                        # work (the PE would otherwise idle on the exp chain)
                        if h == 0:
                            emit_k_chain(c + 1)
                        elif h == 1:
                            emit_q_chain(c + 1, 0)
                        elif h == 2:
                            emit_q_chain(c + 1, 1)
                            emit_q_chain(c + 1, 2)
                        else:
                            emit_q_chain(c + 1, 3)
                    if c > 0:
                        emit_out_tile(c - 1, h)

            emit_norm()
            for sj in range(spc):
                emit_out_tile(n_ch - 1, sj, last=True)

    return nc


# ---------------------------------------------------------------------------
# Host-side prep


_ROPE_PERM = np.concatenate([np.arange(0, HEAD_DIM, 2), np.arange(1, HEAD_DIM, 2)])


def _prep_tables(freq_cis):
    """RoPE tables in [e, s] permuted-half layout, pre-divided by SX*SW.

    rot[0:64]   = q[0:64]*cos   + q[64:128]*(-sin)
    rot[64:128] = q[64:128]*cos + q[0:64]*sin
    """
    fc = np.asarray(freq_cis, dtype=np.float32)
    A = fc[:, :, 0, 0]    # cos  [S, 64]
    Bm = fc[:, :, 0, 1]   # -sin
    C = fc[:, :, 1, 0]    # sin
    Dm = fc[:, :, 1, 1]   # cos
    t1 = np.concatenate([A, Dm], axis=1).T * np.float32(INV_PROJ)  # [128, S]
    t2 = np.concatenate([Bm, C], axis=1).T * np.float32(INV_PROJ)
    return (_bf16(t1), _bf16(t2))


def _prep_tri():
    q = np.arange(128)[None, :]
    p = np.arange(128)[:, None]
    return _bf16(np.where(q >= p, np.float32(1.0), np.float32(0.0)))


def _perm_head_rows(w):
    """Permute rows within each 128-row head block: evens first, odds second."""
    nh = w.shape[0] // HEAD_DIM
    return np.ascontiguousarray(
        w.reshape(nh, HEAD_DIM, -1)[:, _ROPE_PERM, :].reshape(w.shape)
    )


def _bf16(a):
    return np.ascontiguousarray(a.astype(ml_dtypes.bfloat16))


def _hi_lo(a, s):
    """Scaled hi/lo e4m3 split: a*s = hi + lo + O(eps^2)."""
    sa = np.asarray(a, np.float32) * np.float32(s)
    hi = sa.astype(ml_dtypes.float8_e4m3)
    lo = (sa - hi.astype(np.float32)).astype(ml_dtypes.float8_e4m3)
    return np.ascontiguousarray(hi), np.ascontiguousarray(lo)


def _pmajor(a):
    """[T*128, F...] -> [128, T, F...] partition-major layout."""
    t = a.shape[0] // 128
    return np.ascontiguousarray(
        a.reshape(t, 128, *a.shape[1:]).swapaxes(0, 1)
    )


def make_core_inputs(x, freq_cis, wq, wk, wv, wo):
    """Build the 8 per-core input maps."""
    x = np.asarray(x, np.float32)
    wq = np.asarray(wq, np.float32)
    wk = np.asarray(wk, np.float32)
    wv = np.asarray(wv, np.float32)
    wo = np.asarray(wo, np.float32)
    t1, t2 = _prep_tables(freq_cis)
    tri = _prep_tri()
    IQ = HQ * HEAD_DIM

    xT_pm = [_pmajor(x[b].T) for b in range(B)]   # [128, dt, S] f32
    x_hl = [_hi_lo(xp, SX) for xp in xT_pm]

    in_maps = []
    for core in range(8):
        b, g = divmod(core, N_GROUPS)
        wq_g = _perm_head_rows(wq[g * IQ:(g + 1) * IQ])
        wk_g = _perm_head_rows(wk[g * HEAD_DIM:(g + 1) * HEAD_DIM])
        wv_g = wv[g * HEAD_DIM:(g + 1) * HEAD_DIM]
        # [D, IQ] -> [128, dt, IQ] -> [128, HQ, dt, HD] head-major
        wqT = _pmajor(wq_g.T).reshape(128, 16, HQ, HEAD_DIM)
        wqT = np.ascontiguousarray(wqT.swapaxes(1, 2))
        wkT = _pmajor(wk_g.T)
        wvT = _pmajor(wv_g.T)
        woT = _pmajor(wo[:, g * IQ:(g + 1) * IQ].T)
        wqh, wql = _hi_lo(wqT, SW)
        wkh, wkl = _hi_lo(wkT, SW)
        wvh, wvl = _hi_lo(wvT, SW)
        woh, wol = _hi_lo(woT, SW)
        wob = _bf16(woT[:, 2:4, :] * np.float32(SW))
        in_maps.append({
            "xTh": x_hl[b][0],
            "xTl": x_hl[b][1],
            "wqTh": wqh, "wqTl": wql,
            "wkTh": wkh, "wkTl": wkl,
            "wvTh": wvh, "wvTl": wvl,
            "woTh": woh, "woTl": wol,
            "woTb": wob,
            "t1": t1,
            "t2": t2,
            "tri": tri,
        })
    return in_maps


_CACHED_NC = None


def _get_nc():
    global _CACHED_NC
    if _CACHED_NC is None:
        from concourse import bacc

        nc = bacc.Bacc("TRN2", target_bir_lowering=False, debug=False)
        build_attention_core(nc)
        nc.compile()
        _CACHED_NC = nc
    return _CACHED_NC


def kernel(x, freq_cis, wq, wk, wv, wo):
    from concourse.bass_utils import run_bass_kernel_spmd

    nc = _get_nc()
    in_maps = make_core_inputs(x, freq_cis, wq, wk, wv, wo)
    res = run_bass_kernel_spmd(nc, in_maps, list(range(8)))
    out = np.zeros((B, S, DIM), dtype=np.float32)
    for core in range(8):
        b = core // N_GROUPS
        out[b] += res.results[core]["out_partial"].astype(np.float32)
    return out
